# revision 1
# baseline (speedup 1.0000x reference)
import sys

for _p in ('/opt/trn_rl_repo', '/root/.axon_site'):
    if _p not in sys.path:
        sys.path.insert(0, _p)

import numpy as np

B, H, W = 8, 512, 512
K = 3
NCORES = 8
# padded image: 1 zero row/col before, 2 zero rows/cols after (cols padded
# further so shifted views stay in range and rows stay 4B-aligned)
HP, WP = H + 3, W + 8
NBLK = 4          # row blocks of 128 partitions packed along the free dim
AW = 520          # A tile width (Ipad cols 0..519)
DW = 516          # Bv/Dx/Dy tile width

_compiled = None


def _build():
    import concourse.bacc as bacc
    import concourse.mybir as mybir
    from concourse.tile import TileContext, add_dep_helper

    f32, f16 = mybir.dt.float32, mybir.dt.float16
    ALU = mybir.AluOpType
    ACTF = mybir.ActivationFunctionType

    nc = bacc.Bacc("TRN2", target_bir_lowering=False, debug=False,
                   num_devices=NCORES)
    ipad = nc.dram_tensor("ipad", [HP, WP], f16, kind="ExternalInput")
    off = nc.dram_tensor("off", [2 * K * K, H, W], f32, kind="ExternalInput")
    # stack of diag(w_k) matrices used as PE stationary weights
    wdg = nc.dram_tensor("wdg", [128, K * K, 128], f16, kind="ExternalInput")
    out = nc.dram_tensor("out", [H, W], f32, kind="ExternalOutput")

    with TileContext(nc) as tc:
        with (
            tc.tile_pool(name="img", bufs=1) as ip,
            tc.tile_pool(name="l16", bufs=12) as lp,
            tc.tile_pool(name="tmp", bufs=3) as tp,
            tc.tile_pool(name="cst", bufs=1) as cp,
            tc.tile_pool(name="psum", bufs=1, space="PSUM") as pp,
        ):
            wd = cp.tile([128, K * K, 128], f16, name="wd")
            nc.sync.dma_start(out=wd[:], in_=wdg[:])
            psum = pp.tile([128, NBLK, W], f32, name="psum")

            # offsets stream through SWDGE cast-DMA (fp32 HBM -> fp16 SBUF).
            # GpSimd runs no compute: SWDGE descriptor generation is GpSimd
            # ucode and needs the engine idle to sustain full DMA rate.
            # ACT is also kept idle: concurrent big ACT ops starve DVE's
            # SBUF ports (~4x slowdown on overlapping tensor_tensor ops).
            lys, lxs = {}, {}
            lylx_insts = {}

            def load_lylx(k):
                # lx first: it gates m0 at the head of each tap's chain
                lxs[k] = lp.tile([128, NBLK, W], f16, tag="l", name=f"lx{k}")
                i1 = nc.gpsimd.dma_start(
                    out=lxs[k][:],
                    in_=off[2 * k + 1].rearrange("(j p) c -> p j c", p=128))
                lys[k] = lp.tile([128, NBLK, W], f16, tag="l", name=f"ly{k}")
                nc.gpsimd.dma_start(
                    out=lys[k][:],
                    in_=off[2 * k].rearrange("(j p) c -> p j c", p=128))
                lylx_insts[k] = i1

            load_lylx(0)

            # image tiles (fp16 in DRAM) on the two HWDGE rings:
            # A[dy] holds Ipad rows (128j + p + dy + 1); Bv[dy] the same
            # shifted one column (so odd-column views stay 4B-aligned).
            A, Dx, Dy, Dxy = {}, {}, {}, {}

            def load_img(dy):
                A[dy] = ip.tile([128, NBLK, AW], f16, tag=f"A{dy}",
                                name=f"A{dy}")
                eng = nc.sync if dy % 2 == 0 else nc.scalar
                eng.dma_start(
                    out=A[dy][:],
                    in_=ipad[dy + 1:dy + 513, 0:AW].rearrange(
                        "(j p) c -> p j c", p=128))

            for dy in (-1, 0, 1, 2):
                load_img(dy)
            for k in range(1, K * K):
                load_lylx(k)

            def make_dx(dy):
                # Dx = horizontal difference of the padded image
                Dx[dy] = ip.tile([128, NBLK, DW], f16, tag=f"D{dy}",
                                 name=f"D{dy}")
                nc.vector.tensor_tensor(Dx[dy][:], A[dy][:, :, 1:1 + DW],
                                        A[dy][:, :, 0:DW], ALU.subtract)

            def make_dy(j):
                # Dy = vertical difference of the padded image
                Dy[j] = ip.tile([128, NBLK, DW], f16, tag=f"Y{j}",
                                name=f"Y{j}")
                nc.vector.tensor_tensor(Dy[j][:], A[j + 1][:, :, 0:DW],
                                        A[j][:, :, 0:DW], ALU.subtract)

            def make_dxy(j):
                # Dxy = vertical difference of Dx (cross term)
                Dxy[j] = ip.tile([128, NBLK, DW], f16, tag=f"X{j}",
                                 name=f"X{j}")
                nc.vector.tensor_tensor(Dxy[j][:], Dx[j + 1][:],
                                        Dx[j][:], ALU.subtract)

            def iview(dy, q):
                return A[dy][:, :, q:q + W]

            # per tap: v*w_k = w_k*I0 + w_k*m0 + w_k*u
            #   m0 = lx*Dx[ky]
            #   u  = ly*(Dy[ky] + lx*Dxy[ky])
            for k in range(K * K):
                ky, kx = k // K - 1, k % K - 1
                q = kx + 1
                if kx == -1:
                    if ky not in Dx:
                        make_dx(ky)
                    if ky + 1 not in Dx:
                        make_dx(ky + 1)
                    if ky not in Dy:
                        make_dy(ky)
                    if ky not in Dxy:
                        make_dxy(ky)
                ly = lys.pop(k)
                lx = lxs.pop(k)

                t = tp.tile([128, NBLK, W], f16, tag="t", name="t")
                t2 = tp.tile([128, NBLK, W], f16, tag="t2", name="t2")
                t3 = tp.tile([128, NBLK, W], f16, tag="t3", name="t3")
                nc.vector.tensor_tensor(t[:], lx[:], Dx[ky][:, :, q:q + W],
                                        ALU.mult)
                nc.vector.tensor_tensor(t3[:], lx[:], Dxy[ky][:, :, q:q + W],
                                        ALU.mult)
                nc.vector.tensor_tensor(t2[:], t3[:], Dy[ky][:, :, q:q + W],
                                        ALU.add)
                nc.vector.tensor_tensor(t2[:], ly[:], t2[:], ALU.mult)

                wk = wd[:, k, :]
                for j in range(NBLK):
                    nc.tensor.matmul(psum[:, j, :], wk, iview(ky, q)[:, j, :],
                                     start=(k == 0), stop=False)
                    nc.tensor.matmul(psum[:, j, :], wk, t[:, j, :],
                                     start=False, stop=False)
                    nc.tensor.matmul(psum[:, j, :], wk, t2[:, j, :],
                                     start=False, stop=(k == K * K - 1))

            res = cp.tile([128, NBLK, W], f32, name="res")
            nc.scalar.activation(res[:], psum[:], ACTF.Copy)
            nc.sync.dma_start(
                out=out.rearrange("(j p) c -> p j c", p=128), in_=res[:])

    nc.compile()
    return nc


def kernel(input, weight, offset):
    global _compiled
    from concourse.bass_utils import run_bass_kernel_spmd

    if _compiled is None:
        _compiled = _build()
    nc = _compiled

    input = np.asarray(input, dtype=np.float32)
    offset = np.ascontiguousarray(np.asarray(offset, dtype=np.float32))
    w9 = np.asarray(weight, dtype=np.float32).reshape(K * K)
    wdg = np.zeros((128, K * K, 128), np.float16)
    idx = np.arange(128)
    for k in range(K * K):
        wdg[idx, k, idx] = w9[k].astype(np.float16)

    ipad = np.zeros((B, HP, WP), np.float16)
    ipad[:, 1:H + 1, 1:W + 1] = input.astype(np.float16)

    in_maps = [
        {"ipad": ipad[b], "off": offset[b], "wdg": wdg} for b in range(B)
    ]
    res = run_bass_kernel_spmd(nc, in_maps, list(range(NCORES)), trace=False)
    return np.stack([res.results[b]["out"] for b in range(B)], axis=0)

